# revision 16
# baseline (speedup 1.0000x reference)
"""Single-head attention (B=8, S=2048, E=1024, H=128) with softmax + deterministic
dropout, data-parallel over batch across 8 NeuronCores (one batch element per core).

Per-core layout strategy ("transposed attention"):
  - host ships xT = x[b].T           [E, S]  fp16 (contraction dim E on partitions)
  - host ships keepT = keep[b].T     [S, S]  fp16 {0,1} (dropout mask, t-major)
  - qT/kT/vT[h, s] = w.T @ xT        (PE fp16, fp32 PSUM)
  - v natural [t, h] via 16 PE transposes of vT (fp16)
  - attT[t, s] = k[t-chunk] @ qT     (PE fp16; lhsT = kT chunk, rhs = qT slice)
  - expT = exp(attT * E^-0.5)        (ACT, PSUM -> fp16 SBUF)
  - denomT[1, s] += ones.T @ expT    (PE fp16, M=1, accumulated over t-chunks)
  - attd = expT * keepT              (DVE fp16, 2x mode)
  - outT[h, s] += v[t-chunk].T @ attd  (PE fp16, fp32 PSUM accumulation)
  - normalize by 1/(0.9*denom) fused into the final PSUM->SBUF copy (ACT
    per-partition scale) after PE-transposing outT back to natural [s, h].

Precision: fp16 rounding on x/w/q/k contributes only ~3e-5 to the softmax
logits (their absolute scale is ~0.1 after the E^-0.5 scaling); the fp16
value path (v, exp, attd) dominates at ~2-4e-4 L2 on the output, with all
contractions accumulated in fp32 PSUM.

The q projection runs e-major (accumulating all four 512-wide column groups
at once) so the PE starts as soon as the first xT chunk lands instead of
waiting for the whole xT transfer.
"""

import sys

for _p in ("/opt/trn_rl_repo",):
    if _p not in sys.path:
        sys.path.append(_p)

import numpy as np

B, S, E, H = 8, 2048, 1024, 128
DROP_P = 0.1
P = 128

_program_cache = {}


def _build_program(S=S, E=E):
    key = (S, E)
    if key in _program_cache:
        return _program_cache[key]
    NT = S // P   # t-chunks
    NE = E // P   # e-chunks
    SG = 512      # s-group width (one fp32 PSUM bank)
    NSG = S // SG
    NC4 = SG // P

    import concourse.bass as bass  # noqa: F401
    import concourse.mybir as mybir
    import concourse.tile as tile
    from concourse import bacc
    from concourse.masks import make_identity

    f32 = mybir.dt.float32
    f16 = mybir.dt.float16
    Exp = mybir.ActivationFunctionType.Exp
    Copy = mybir.ActivationFunctionType.Copy
    SCALE = float(E) ** -0.5

    nc = bacc.Bacc("TRN2", target_bir_lowering=False, debug=False)
    xT_d = nc.dram_tensor("xT", [E, S], f16, kind="ExternalInput").ap()
    keepT_d = nc.dram_tensor("keepT", [S, S], f16, kind="ExternalInput").ap()
    wq_d = nc.dram_tensor("wq", [E, H], f16, kind="ExternalInput").ap()
    wk_d = nc.dram_tensor("wk", [E, H], f16, kind="ExternalInput").ap()
    wv_d = nc.dram_tensor("wv", [E, H], f16, kind="ExternalInput").ap()
    out_d = nc.dram_tensor("out", [S, H], f32, kind="ExternalOutput").ap()

    xT_r = xT_d.rearrange("(eo p) s -> p eo s", p=P)
    w_rs = [w.rearrange("(eo p) h -> p eo h", p=P) for w in (wq_d, wk_d, wv_d)]
    # keepT viewed as [p, t_chunk, s] so one DMA loads a whole s-column block
    keepT_r = keepT_d.rearrange("(to p) s -> p to s", p=P)

    with tile.TileContext(nc) as tc:
        with (
            tc.tile_pool(name="consts", bufs=1) as consts,
            tc.tile_pool(name="xw", bufs=1) as xw_pool,
            tc.tile_pool(name="qkv", bufs=1) as qkv_pool,
        ):
            identity = consts.tile([P, P], f32)
            make_identity(nc, identity)
            identity16 = consts.tile([P, P], f16)
            nc.any.tensor_copy(identity16, identity)
            ones_t = consts.tile([P, 1], f16)
            nc.vector.memset(ones_t, 1.0)

            # -------- load weights first (small, gates the first matmul),
            # then x^T as per-e tiles so projections start as chunks land ----
            w_sb = xw_pool.tile([P, 3, NE, H], f16)
            for j in range(3):
                nc.sync.dma_start(w_sb[:, j], w_rs[j])
            xT_es = []
            for e in range(NE):
                xe = xw_pool.tile([P, S], f16, tag=f"x{e}", name=f"x{e}")
                nc.sync.dma_start(xe, xT_r[:, e, :])
                xT_es.append(xe)

            # -------- projections: qT/kT [H, S] f16; vT -> v natural f16 ----
            qkT_sb = qkv_pool.tile([P, 2, S], f16)  # [h, (q|k), s]
            vT_sb = qkv_pool.tile([P, S], f16)
            v_sb = qkv_pool.tile([P, NT, H], f16)   # v natural: [t_in, t_chunk, h]
            with tc.tile_pool(name="proj_ps", bufs=3, space="PSUM") as proj_ps:
                for j in (0, 1, 2):
                    for c in range(S // SG):
                        ps = proj_ps.tile([P, SG], f32, tag="proj")
                        for e in range(NE):
                            nc.tensor.matmul(
                                ps,
                                w_sb[:, j, e, :],
                                xT_es[e][:, c * SG:(c + 1) * SG],
                                start=(e == 0),
                                stop=(e == NE - 1),
                            )
                        if j < 2:
                            nc.any.tensor_copy(qkT_sb[:, j, c * SG:(c + 1) * SG], ps)
                        else:
                            nc.any.tensor_copy(vT_sb[:, c * SG:(c + 1) * SG], ps)
                # v natural via PE transpose of vT (fp16, exact for fp16 data)
                for t in range(NT):
                    ps_v = proj_ps.tile([P, SG], f16, tag="proj", name="ps_v")
                    nc.tensor.transpose(
                        ps_v[:, 0:P], vT_sb[:, t * P:(t + 1) * P], identity16
                    )
                    nc.any.tensor_copy(v_sb[:, t, :], ps_v[:, 0:P])

            # -------- main attention loop over s-groups --------
            with (
                tc.tile_pool(name="att_ps", bufs=3, space="PSUM") as att_ps,
                tc.tile_pool(name="out_ps", bufs=2, space="PSUM") as out_ps,
                tc.tile_pool(name="den_ps", bufs=1, space="PSUM") as den_ps,
                tc.tile_pool(name="tr2_ps", bufs=2, space="PSUM") as tr2_ps,
                tc.tile_pool(name="sb", bufs=3) as sb_pool,
                tc.tile_pool(name="sb2", bufs=2) as sb2_pool,
            ):
                for sg in range(NSG):
                    s_lo = sg * SG
                    s_sl = slice(s_lo, s_lo + SG)
                    psum_out = out_ps.tile([P, SG], f32, tag="out")
                    psum_den = den_ps.tile([1, SG], f32, tag="den")
                    expTs = {}
                    attds = {}

                    def emit_front(t, s_sl=s_sl, expTs=expTs, attds=attds):
                        psum_att = att_ps.tile([P, SG], f32, tag="att", name=f"att{t}")
                        nc.tensor.matmul(
                            psum_att,
                            qkT_sb[:, 1, t * P:(t + 1) * P],  # kT chunk [H, 128]
                            qkT_sb[:, 0, s_sl],               # qT slice [H, 512]
                            start=True,
                            stop=True,
                        )
                        expT = sb_pool.tile([P, SG], f16, tag="exp", name=f"exp{t}")
                        nc.scalar.activation(expT, psum_att, Exp, scale=SCALE)
                        keep_sb = sb_pool.tile([P, SG], f16, tag="keep", name=f"keep{t}")
                        nc.sync.dma_start(keep_sb, keepT_d[t * P:(t + 1) * P, s_sl])
                        attd = sb_pool.tile([P, SG], f16, tag="attd", name=f"attd{t}")
                        nc.vector.tensor_mul(out=attd, in0=expT, in1=keep_sb)
                        expTs[t] = expT
                        attds[t] = attd

                    def emit_back(t, psum_den=psum_den, psum_out=psum_out,
                                  expTs=expTs, attds=attds):
                        nc.tensor.matmul(
                            psum_den,
                            ones_t,
                            expTs.pop(t),
                            start=(t == 0),
                            stop=(t == NT - 1),
                        )
                        nc.tensor.matmul(
                            psum_out,
                            v_sb[:, t, :],
                            attds.pop(t),
                            start=(t == 0),
                            stop=(t == NT - 1),
                        )

                    # software pipeline: back-stage ops run one iteration behind
                    # the att matmul so PE never waits on ACT/DVE results.
                    for t in range(NT):
                        emit_front(t)
                        if t >= 1:
                            emit_back(t - 1)
                    emit_back(NT - 1)

                    # denominator -> natural-layout 1/(0.9*den) chunks [s_in, 1]
                    den_sb = sb2_pool.tile([1, SG], f32, tag="den_sb")
                    nc.scalar.mul(den_sb, psum_den, 1.0 - DROP_P)
                    outT_sb = sb2_pool.tile([P, SG], f32, tag="outT")
                    nc.any.tensor_copy(outT_sb, psum_out)
                    recip_nat = sb2_pool.tile([P, NC4], f32, tag="recip")
                    for c in range(NC4):
                        ps_rt = tr2_ps.tile([P, P], f32, tag="tr", name="ps_rt")
                        ps_r = ps_rt[:, 0:1]
                        nc.tensor.transpose(
                            ps_r, den_sb[:, c * P:(c + 1) * P], identity[0:1, 0:1]
                        )
                        nc.vector.reciprocal(recip_nat[:, c:c + 1], ps_r)
                    # transpose outT back to natural [s, h] and scale by recip
                    for c in range(NC4):
                        ps_ot = tr2_ps.tile([P, P], f32, tag="tr", name="ps_ot")
                        ps_o = ps_ot[:, 0:P]
                        nc.tensor.transpose(
                            ps_o, outT_sb[:, c * P:(c + 1) * P], identity
                        )
                        out_nat = sb2_pool.tile([P, H], f32, tag="out_nat")
                        nc.scalar.activation(
                            out_nat, ps_o, Copy, scale=recip_nat[:, c:c + 1]
                        )
                        row = s_lo + c * P
                        nc.sync.dma_start(out_d[row:row + P, :], out_nat)

    nc.compile()
    _program_cache[key] = nc
    return nc


def kernel(x, wq, wk, wv, drop_u):
    from concourse import bass_utils

    x = np.asarray(x)
    wq = np.asarray(wq)
    wk = np.asarray(wk)
    wv = np.asarray(wv)
    drop_u = np.asarray(drop_u)

    nc = _build_program()
    in_maps = build_in_maps(x, wq, wk, wv, drop_u)
    res = bass_utils.run_bass_kernel_spmd(
        nc, in_maps, core_ids=list(range(B)), trace=False
    )
    return np.stack([res.results[b]["out"] for b in range(B)], axis=0)


def build_in_maps(x, wq, wk, wv, drop_u):
    wq16 = np.asarray(wq).astype(np.float16)
    wk16 = np.asarray(wk).astype(np.float16)
    wv16 = np.asarray(wv).astype(np.float16)
    in_maps = []
    for b in range(B):
        xT = np.ascontiguousarray(x[b].T).astype(np.float16)
        keepT = np.ascontiguousarray(
            (drop_u[b].T >= np.float32(DROP_P)).astype(np.float16)
        )
        in_maps.append(
            {"xT": xT, "keepT": keepT, "wq": wq16, "wk": wk16, "wv": wv16}
        )
    return in_maps


# revision 25
# speedup vs baseline: 1.3630x; 1.3630x over previous
"""Single-head attention (B=8, S=2048, E=1024, H=128) with softmax + deterministic
dropout, data-parallel over batch across 8 NeuronCores (one batch element per core).

Per-core layout strategy ("transposed attention"):
  - host ships xT = x[b].T           [E, S]  fp16 (contraction dim E on partitions)
  - host ships keepT = keep[b].T     [S, S]  fp16 {0,1} (dropout mask, t-major)
  - qT/kT/vT[h, s] = w.T @ xT        (PE fp16, fp32 PSUM)
  - v natural [t, h] via 16 PE transposes of vT (fp16)
  - attT[t, s] = k[t-chunk] @ qT     (PE fp16; lhsT = kT chunk, rhs = qT slice)
  - expT = exp(attT * E^-0.5)        (ACT, PSUM -> fp16 SBUF)
  - denomT[1, s] += ones.T @ expT    (PE fp16, M=1, accumulated over t-chunks)
  - attd = expT * keepT              (DVE fp16, 2x mode)
  - outT[h, s] += v[t-chunk].T @ attd  (PE fp16, fp32 PSUM accumulation)
  - normalize by 1/(0.9*denom) fused into the final PSUM->SBUF copy (ACT
    per-partition scale) after PE-transposing outT back to natural [s, h].

Precision: fp16 rounding on x/w/q/k contributes only ~3e-5 to the softmax
logits (their absolute scale is ~0.1 after the E^-0.5 scaling); the fp16
value path (v, exp, attd) dominates at ~2-4e-4 L2 on the output, with all
contractions accumulated in fp32 PSUM.

The q projection runs e-major (accumulating all four 512-wide column groups
at once) so the PE starts as soon as the first xT chunk lands instead of
waiting for the whole xT transfer.
"""

import sys

for _p in ("/opt/trn_rl_repo",):
    if _p not in sys.path:
        sys.path.append(_p)

import numpy as np

B, S, E, H = 8, 2048, 1024, 128
DROP_P = 0.1
P = 128

_program_cache = {}


def _build_program(S=S, E=E):
    key = (S, E)
    if key in _program_cache:
        return _program_cache[key]
    NT = S // P   # t-chunks
    NE = E // P   # e-chunks
    SG = 512      # s-group width (one fp32 PSUM bank)
    NSG = S // SG
    NC4 = SG // P

    import concourse.bass as bass  # noqa: F401
    import concourse.mybir as mybir
    import concourse.tile as tile
    from concourse import bacc
    from concourse.masks import make_identity

    f32 = mybir.dt.float32
    f16 = mybir.dt.float16
    Exp = mybir.ActivationFunctionType.Exp
    Copy = mybir.ActivationFunctionType.Copy
    SCALE = float(E) ** -0.5

    nc = bacc.Bacc("TRN2", target_bir_lowering=False, debug=False)
    xT_d = nc.dram_tensor("xT", [E, S], f16, kind="ExternalInput").ap()
    keepT_d = nc.dram_tensor("keepT", [S, S], f16, kind="ExternalInput").ap()
    wq_d = nc.dram_tensor("wq", [E, H], f16, kind="ExternalInput").ap()
    wk_d = nc.dram_tensor("wk", [E, H], f16, kind="ExternalInput").ap()
    wv_d = nc.dram_tensor("wv", [E, H], f16, kind="ExternalInput").ap()
    out_d = nc.dram_tensor("out", [S, H], f32, kind="ExternalOutput").ap()

    xT_r = xT_d.rearrange("(eo p) s -> p eo s", p=P)
    w_rs = [w.rearrange("(eo p) h -> p eo h", p=P) for w in (wq_d, wk_d, wv_d)]
    # keepT viewed as [p, t_chunk, s] so one DMA loads a whole s-column block
    keepT_r = keepT_d.rearrange("(to p) s -> p to s", p=P)

    with tile.TileContext(nc) as tc:
        with (
            tc.tile_pool(name="consts", bufs=1) as consts,
            tc.tile_pool(name="xw", bufs=1) as xw_pool,
            tc.tile_pool(name="qkv", bufs=1) as qkv_pool,
        ):
            identity = consts.tile([P, P], f32)
            make_identity(nc, identity)
            identity16 = consts.tile([P, P], f16)
            nc.any.tensor_copy(identity16, identity)
            ones_t = consts.tile([P, 1], f16)
            nc.vector.memset(ones_t, 1.0)
            sel4 = consts.tile([P, 1], f16)
            nc.vector.memset(sel4, 0.0)
            for j in range(4):
                nc.vector.memset(sel4[32 * j:32 * j + 1, :], 1.0)

            # -------- load weights first (small, gates the first matmul),
            # then x^T as per-e tiles so projections start as chunks land ----
            w_js = []
            for j in range(3):
                wj = xw_pool.tile([P, NE, H], f16, tag=f"w{j}", name=f"w{j}")
                w_js.append(wj)
            # order issues so the first q matmul's inputs (wq, x0) land first
            nc.sync.dma_start(w_js[0], w_rs[0])
            xT_es = []
            for e in range(NE):
                xe = xw_pool.tile([P, S], f16, tag=f"x{e}", name=f"x{e}")
                xT_es.append(xe)
            nc.sync.dma_start(xT_es[0], xT_r[:, 0, :])
            nc.sync.dma_start(w_js[1], w_rs[1])
            nc.sync.dma_start(w_js[2], w_rs[2])
            for e in range(1, NE):
                nc.sync.dma_start(xT_es[e], xT_r[:, e, :])

            # -------- projections: qT/kT [H, S] f16; vT -> v natural f16 ----
            qkT_sb = qkv_pool.tile([P, 2, S], f16)  # [h, (q|k), s]
            vT_sb = qkv_pool.tile([P, S], f16)
            v_sb = qkv_pool.tile([P, NT, H], f16)   # v natural: [t_in, t_chunk, h]
            with tc.tile_pool(name="proj_ps", bufs=3, space="PSUM") as proj_ps:
                # q projection e-major: starts on the first xT chunk, all four
                # column-group accumulators live at once (4 PSUM banks).
                ps_qs = [proj_ps.tile([P, SG], f32, tag=f"pq{c}", name=f"pq{c}",
                                      bufs=1) for c in range(S // SG)]
                for e in range(NE):
                    for c in range(S // SG):
                        nc.tensor.matmul(
                            ps_qs[c],
                            w_js[0][:, e, :],
                            xT_es[e][:, c * SG:(c + 1) * SG],
                            start=(e == 0),
                            stop=(e == NE - 1),
                        )
                for c in range(S // SG):
                    nc.any.tensor_copy(qkT_sb[:, 0, c * SG:(c + 1) * SG], ps_qs[c])
                for j in (1, 2):
                    for c in range(S // SG):
                        ps = proj_ps.tile([P, SG], f32, tag="proj")
                        for e in range(NE):
                            nc.tensor.matmul(
                                ps,
                                w_js[j][:, e, :],
                                xT_es[e][:, c * SG:(c + 1) * SG],
                                start=(e == 0),
                                stop=(e == NE - 1),
                            )
                        if j == 1:
                            nc.any.tensor_copy(qkT_sb[:, 1, c * SG:(c + 1) * SG], ps)
                        else:
                            nc.any.tensor_copy(vT_sb[:, c * SG:(c + 1) * SG], ps)
                # v natural via PE transpose of vT (fp16, exact for fp16 data)
                for t in range(NT):
                    ps_v = proj_ps.tile([P, SG], f16, tag="proj", name="ps_v")
                    nc.tensor.transpose(
                        ps_v[:, 0:P], vT_sb[:, t * P:(t + 1) * P], identity16
                    )
                    nc.any.tensor_copy(v_sb[:, t, :], ps_v[:, 0:P])

            # -------- main attention loop over s-groups --------
            with (
                tc.tile_pool(name="att_ps", bufs=4, space="PSUM") as att_ps,
                tc.tile_pool(name="out_ps", bufs=1, space="PSUM") as out_ps,
                tc.tile_pool(name="den_ps", bufs=1, space="PSUM") as den_ps,
                tc.tile_pool(name="tr2_ps", bufs=2, space="PSUM") as tr2_ps,
                tc.tile_pool(name="keep_pool", bufs=2) as keep_pool,
                tc.tile_pool(name="sb", bufs=6) as sb_pool,
                tc.tile_pool(name="sb2", bufs=2) as sb2_pool,
            ):
                keeps = {}

                def fetch_keep(sg):
                    keeps[sg] = keep_pool.tile([P, NT, SG], f16, tag="keep",
                                               name=f"keep{sg}")
                    nc.sync.dma_start(
                        keeps[sg], keepT_r[:, :, sg * SG:(sg + 1) * SG])

                fetch_keep(0)
                for sg in range(NSG):
                    s_lo = sg * SG
                    s_sl = slice(s_lo, s_lo + SG)
                    if sg + 1 < NSG:
                        fetch_keep(sg + 1)
                    keep_sg = keeps.pop(sg)
                    psum_out = out_ps.tile([P, SG], f32, tag="out")
                    psum_den = den_ps.tile([P, SG], f32, tag="den")
                    nc.vector.memset(psum_den, 0.0)
                    expTs = {}
                    attds = {}

                    def emit_front(t, s_sl=s_sl, keep_sg=keep_sg,
                                   expTs=expTs, attds=attds):
                        psum_att = att_ps.tile([P, SG], f32, tag="att", name=f"att{t}")
                        nc.tensor.matmul(
                            psum_att,
                            qkT_sb[:, 1, t * P:(t + 1) * P],  # kT chunk [H, 128]
                            qkT_sb[:, 0, s_sl],               # qT slice [H, 512]
                            start=True,
                            stop=True,
                        )
                        expT = sb_pool.tile([P, SG], f16, tag="exp", name=f"exp{t}")
                        nc.scalar.activation(expT, psum_att, Exp, scale=SCALE)
                        attd = sb_pool.tile([P, SG], f16, tag="attd", name=f"attd{t}")
                        nc.vector.tensor_mul(out=attd, in0=expT, in1=keep_sg[:, t, :])
                        expTs[t] = expT
                        attds[t] = attd

                    def emit_den_wave(k, psum_den=psum_den, expTs=expTs):
                        # 4 M=1 matmuls packed into distinct 32-column groups of
                        # the PE array -> they run concurrently (distinct col_grp)
                        for j in range(4):
                            t = 4 * k + j
                            nc.tensor.matmul(
                                psum_den[32 * j:32 * j + 1, :],
                                ones_t,
                                expTs.pop(t),
                                start=(k == 0),
                                stop=(k == NT // 4 - 1),
                                tile_position=(0, 32 * j),
                            )

                    def emit_out(t, psum_out=psum_out, attds=attds):
                        nc.tensor.matmul(
                            psum_out,
                            v_sb[:, t, :],
                            attds.pop(t),
                            start=(t == 0),
                            stop=(t == NT - 1),
                        )

                    # software pipeline in waves of 4: den matmuls of a wave are
                    # emitted back-to-back so their column-group packing overlaps.
                    NW = NT // 4
                    for k in range(NW):
                        for t in range(4 * k, 4 * k + 4):
                            emit_front(t)
                        if k >= 1:
                            emit_den_wave(k - 1)
                            for t in range(4 * (k - 1), 4 * k):
                                emit_out(t)
                    emit_den_wave(NW - 1)
                    for t in range(4 * (NW - 1), NT):
                        emit_out(t)

                    # combine the 4 column-group partial denominators via a
                    # select-vector matmul, then -> natural-layout 1/(0.9*den)
                    den_all_sb = sb2_pool.tile([P, SG], f16, tag="den_all")
                    nc.vector.tensor_copy(den_all_sb, psum_den)
                    psum_drow_t = tr2_ps.tile([P, SG], f32, tag="tr", name="psum_drow")
                    psum_drow = psum_drow_t[0:1, :]
                    nc.tensor.matmul(
                        psum_drow, sel4, den_all_sb,
                        start=True, stop=True,
                    )
                    den_sb = sb2_pool.tile([1, SG], f32, tag="den_sb")
                    nc.scalar.mul(den_sb, psum_drow, 1.0 - DROP_P)
                    outT_sb = sb2_pool.tile([P, SG], f32, tag="outT")
                    nc.any.tensor_copy(outT_sb, psum_out)
                    recip_nat = sb2_pool.tile([P, NC4], f32, tag="recip")
                    for c in range(NC4):
                        ps_rt = tr2_ps.tile([P, SG], f32, tag="tr", name="ps_rt")
                        ps_r = ps_rt[:, 0:1]
                        nc.tensor.transpose(
                            ps_r, den_sb[:, c * P:(c + 1) * P], identity[0:1, 0:1]
                        )
                        nc.vector.reciprocal(recip_nat[:, c:c + 1], ps_r)
                    # transpose outT back to natural [s, h] and scale by recip
                    for c in range(NC4):
                        ps_ot = tr2_ps.tile([P, SG], f32, tag="tr", name="ps_ot")
                        ps_o = ps_ot[:, 0:P]
                        nc.tensor.transpose(
                            ps_o, outT_sb[:, c * P:(c + 1) * P], identity
                        )
                        out_nat = sb2_pool.tile([P, H], f32, tag="out_nat")
                        nc.vector.tensor_scalar_mul(
                            out_nat, ps_o, recip_nat[:, c:c + 1]
                        )
                        row = s_lo + c * P
                        nc.sync.dma_start(out_d[row:row + P, :], out_nat)

    nc.compile()
    _program_cache[key] = nc
    return nc


def kernel(x, wq, wk, wv, drop_u):
    from concourse import bass_utils

    x = np.asarray(x)
    wq = np.asarray(wq)
    wk = np.asarray(wk)
    wv = np.asarray(wv)
    drop_u = np.asarray(drop_u)

    nc = _build_program()
    in_maps = build_in_maps(x, wq, wk, wv, drop_u)
    res = bass_utils.run_bass_kernel_spmd(
        nc, in_maps, core_ids=list(range(B)), trace=False
    )
    return np.stack([res.results[b]["out"] for b in range(B)], axis=0)


def build_in_maps(x, wq, wk, wv, drop_u):
    wq16 = np.asarray(wq).astype(np.float16)
    wk16 = np.asarray(wk).astype(np.float16)
    wv16 = np.asarray(wv).astype(np.float16)
    in_maps = []
    for b in range(B):
        xT = np.ascontiguousarray(x[b].T).astype(np.float16)
        keepT = np.ascontiguousarray(
            (drop_u[b].T >= np.float32(DROP_P)).astype(np.float16)
        )
        in_maps.append(
            {"xT": xT, "keepT": keepT, "wq": wq16, "wk": wk16, "wv": wv16}
        )
    return in_maps
